# revision 6
# baseline (speedup 1.0000x reference)
"""Trainium2 Bass kernel for CustomHyperSemanticMessagePassing (hypergraph
multi-head single-query attention message passing).

Math (reference):
  Wh = x @ w_lin ; We = edge_attr @ w_e
  q  = (Wh @ w_q + b_q)/sqrt(dh)               per node, [N,H,dh]
  k_p = (Wh[u] + We[e]) @ w_k + b_k            per pair (v,e,u)
  v_p = Wh[u] @ w_v + b_v
  scores_p = <q[v], k_p> per head ; segmented softmax over each node v's pairs
  out = relu(segsum(alpha * v_p) @ w_o + b_o)

Kernel strategy (8 NeuronCores, SPMD, no collectives):
  * Algebraic refactor: fold w_lin into the q/k/v projections so no per-pair
    matmuls are needed:
      KV table  [N,256] = [x @ (w_lin w_k) | x @ (w_lin w_v) + b_v]
      KE table  [E,128] =  edge_attr @ (w_e w_k) + b_k
      q table   [Nloc,128] = (x_loc @ (w_lin w_q) + b_q)/4
    k_p = KV[u,:128] + KE[e]; v_p = KV[u,128:].
  * Every core builds the full KV/KE tables (the table must land in every
    core's HBM anyway; recompute is cheaper than collectives) and the q table
    for its own 1/8 node slab.
  * Pairs are routed (host side) to the core owning v, sorted by owner, and
    bin-packed into blocks of <=128 owner nodes / <=1024 pairs. Per 128-pair
    tile the device gathers KV/KE rows with indirect DMA, expands q via a
    one-hot matmul, computes scores with vector ops, exp on ScalarE (scores
    are O(1), max-subtraction provably unnecessary), and segment-reduces
    numerator+denominator with a one-hot scatter matmul accumulated in PSUM.
  * Host un-permutes the per-core block-major outputs into the final [N,128].
"""

import math
import numpy as np
from contextlib import ExitStack

import concourse.bass as bass
import concourse.bacc as bacc
import concourse.tile as tile
import concourse.mybir as mybir
from concourse.bass_utils import run_bass_kernel_spmd
from concourse.masks import make_identity

F32 = mybir.dt.float32
I32 = mybir.dt.int32

N, E, D, ED, H = 100000, 50000, 128, 64, 8
DH = D // H
NC = 8
NLOC = N // NC
P128 = 128
TPB = 8                  # 128-pair tiles per block
CAP = TPB * P128         # pairs per block
CAP_NODES = 128          # owner nodes per block (M-matrix columns)
PAD_BIAS = -30000.0      # additive score bias for padding pairs -> exp == 0

NTX = (N + P128 - 1) // P128        # 782 x tiles
PADN = NTX * P128
NTE = (E + P128 - 1) // P128        # 391 edge tiles
PADE = NTE * P128
NTQ = (NLOC + P128 - 1) // P128     # 98 q tiles per core
PADQ = NTQ * P128


# ----------------------------------------------------------------------------
# host-side routing
# ----------------------------------------------------------------------------

def _pack_nodes(cnt, nblk):
    """Worst-fit-decreasing packing of nodes into nblk blocks with
    <=CAP_NODES nodes and <=CAP pairs each. Returns list of node-id lists or
    None if it does not fit."""
    import heapq
    order = np.argsort(-cnt, kind="stable")
    heap = [(-CAP, bi) for bi in range(nblk)]
    heapq.heapify(heap)
    nodes = [[] for _ in range(nblk)]
    for nid in order:
        c = int(cnt[nid])
        placed = False
        while heap:
            negrem, bi = heapq.heappop(heap)
            rem = -negrem
            if rem < c:
                heapq.heappush(heap, (negrem, bi))
                break
            nodes[bi].append(nid)
            if len(nodes[bi]) < CAP_NODES:
                heapq.heappush(heap, (-(rem - c), bi))
            placed = True
            break
        if not placed:
            return None
    return nodes


def _route(owners, pair_e, pair_u):
    """Sort pairs by owner, split per core, pack blocks, build device arrays.

    Returns (nblk, per_core_arrays, node_map) where node_map[c] maps output
    row -> global node id (-1 for padding)."""
    perm = np.argsort(owners, kind="stable")
    o_s = owners[perm]
    e_s = pair_e[perm]
    u_s = pair_u[perm]
    bounds = np.searchsorted(o_s, np.arange(NC + 1) * NLOC)

    packs = []
    nblk = 0
    for c in range(NC):
        lo, hi = int(bounds[c]), int(bounds[c + 1])
        loc = o_s[lo:hi] - c * NLOC
        cnt = np.bincount(loc, minlength=NLOC)
        starts = np.zeros(NLOC + 1, np.int64)
        np.cumsum(cnt, out=starts[1:])
        nb = max(math.ceil(NLOC / CAP_NODES), math.ceil((hi - lo) / CAP))
        while True:
            nodes = _pack_nodes(cnt, nb)
            if nodes is not None:
                break
            nb += 1
        packs.append((lo, nodes, cnt, starts))
        nblk = max(nblk, nb)

    per_core = []
    node_map = np.full((NC, 0), -1, np.int64)
    maps = []
    for c in range(NC):
        lo, nodes, cnt, starts = packs[c]
        ints = np.zeros((nblk, P128, 17), np.int32)
        floats = np.zeros((nblk, P128, 16), np.float32)
        orow = np.zeros((nblk, 1, CAP), np.float32)
        nmap = np.full(nblk * P128, -1, np.int64)
        for b, blk in enumerate(nodes):
            fu = np.zeros(CAP, np.int32)
            fe = np.zeros(CAP, np.int32)
            frel = np.zeros(CAP, np.float32)
            fbias = np.full(CAP, PAD_BIAS, np.float32)
            qidx = np.zeros(P128, np.int32)
            pos = 0
            for j, nid in enumerate(blk):
                s0 = lo + int(starts[nid])
                k = int(cnt[nid])
                fu[pos:pos + k] = u_s[s0:s0 + k]
                fe[pos:pos + k] = e_s[s0:s0 + k]
                frel[pos:pos + k] = j
                fbias[pos:pos + k] = 0.0
                qidx[j] = nid
                nmap[b * P128 + j] = c * NLOC + nid
                pos += k
            ints[b, :, 0:8] = fu.reshape(TPB, P128).T
            ints[b, :, 8:16] = fe.reshape(TPB, P128).T
            ints[b, :, 16] = qidx
            floats[b, :, 0:8] = frel.reshape(TPB, P128).T
            floats[b, :, 8:16] = fbias.reshape(TPB, P128).T
            orow[b, 0, :] = frel
        per_core.append((ints, floats, orow))
        maps.append(nmap)
    return nblk, per_core, np.stack(maps)


# ----------------------------------------------------------------------------
# device program
# ----------------------------------------------------------------------------

def _build_nc(nblk):
    nc = bacc.Bacc()

    xT = nc.declare_dram_parameter("xT", [P128, PADN], F32, isOutput=False)
    xqT = nc.declare_dram_parameter("xqT", [P128, PADQ], F32, isOutput=False)
    eaT = nc.declare_dram_parameter("eaT", [ED, PADE], F32, isOutput=False)
    w_linT = nc.declare_dram_parameter("w_linT", [D, D], F32, isOutput=False)
    w_eT = nc.declare_dram_parameter("w_eT", [D, ED], F32, isOutput=False)
    w_q = nc.declare_dram_parameter("w_q", [D, D], F32, isOutput=False)
    w_k = nc.declare_dram_parameter("w_k", [D, D], F32, isOutput=False)
    w_v = nc.declare_dram_parameter("w_v", [D, D], F32, isOutput=False)
    w_o = nc.declare_dram_parameter("w_o", [D, D], F32, isOutput=False)
    bk_m = nc.declare_dram_parameter("bk_m", [P128, D], F32, isOutput=False)
    bv_m = nc.declare_dram_parameter("bv_m", [P128, D], F32, isOutput=False)
    bq_m = nc.declare_dram_parameter("bq_m", [P128, D], F32, isOutput=False)
    bo_m = nc.declare_dram_parameter("bo_m", [P128, D], F32, isOutput=False)
    ints_p = nc.declare_dram_parameter("ints_p", [nblk, P128, 17], I32, isOutput=False)
    floats_p = nc.declare_dram_parameter("floats_p", [nblk, P128, 16], F32, isOutput=False)
    orow_p = nc.declare_dram_parameter("orow_p", [nblk, 1, CAP], F32, isOutput=False)
    out = nc.declare_dram_parameter("out", [nblk * P128, D], F32, isOutput=True)

    kv_d = nc.dram_tensor("kv_d", [PADN, 2 * D], F32)
    ke_d = nc.dram_tensor("ke_d", [PADE, D], F32)
    q_d = nc.dram_tensor("q_d", [PADQ, D], F32)

    with ExitStack() as ctx:
        tc = ctx.enter_context(tile.TileContext(nc))
        consts = ctx.enter_context(tc.tile_pool(name="consts", bufs=1))

        ident = consts.tile([P128, P128], F32)
        make_identity(nc, ident[:])
        iota_row_i = consts.tile([P128, P128], I32)
        nc.gpsimd.iota(iota_row_i[:], pattern=[[1, P128]], base=0, channel_multiplier=0)
        iota_row = consts.tile([P128, P128], F32)
        nc.vector.tensor_copy(iota_row[:], iota_row_i[:])
        iota_col_i = consts.tile([P128, 1], I32)
        nc.gpsimd.iota(iota_col_i[:], pattern=[[0, 1]], base=0, channel_multiplier=1)
        iota_col = consts.tile([P128, 1], F32)
        nc.vector.tensor_copy(iota_col[:], iota_col_i[:])
        ones_row = consts.tile([1, P128], F32)
        nc.vector.memset(ones_row[:], 1.0)

        # ---- load weights / fold projections --------------------------------
        wlt_sb = consts.tile([D, D], F32)
        nc.sync.dma_start(out=wlt_sb[:], in_=w_linT[:, :])
        wet_sb = consts.tile([D, ED], F32)
        nc.sync.dma_start(out=wet_sb[:], in_=w_eT[:, :])
        wq_sb = consts.tile([D, D], F32)
        nc.sync.dma_start(out=wq_sb[:], in_=w_q[:, :])
        wk_sb = consts.tile([D, D], F32)
        nc.sync.dma_start(out=wk_sb[:], in_=w_k[:, :])
        wv_sb = consts.tile([D, D], F32)
        nc.sync.dma_start(out=wv_sb[:], in_=w_v[:, :])
        wo_sb = consts.tile([D, D], F32)
        nc.sync.dma_start(out=wo_sb[:], in_=w_o[:, :])
        bk_sb = consts.tile([P128, D], F32)
        nc.sync.dma_start(out=bk_sb[:], in_=bk_m[:, :])
        bv_sb = consts.tile([P128, D], F32)
        nc.sync.dma_start(out=bv_sb[:], in_=bv_m[:, :])
        bq_raw = consts.tile([P128, D], F32)
        nc.sync.dma_start(out=bq_raw[:], in_=bq_m[:, :])
        bo_sb = consts.tile([P128, D], F32)
        nc.sync.dma_start(out=bo_sb[:], in_=bo_m[:, :])
        bq_sb = consts.tile([P128, D], F32)
        nc.scalar.mul(out=bq_sb[:], in_=bq_raw[:], mul=1.0 / math.sqrt(DH))

        w_cat = consts.tile([D, 2 * D], F32)      # [w_lin w_k | w_lin w_v]
        w_lq = consts.tile([D, D], F32)           # (w_lin w_q)/sqrt(dh)
        w_ek = consts.tile([ED, D], F32)          # w_e w_k

        with tc.tile_pool(name="ps0", bufs=1, space="PSUM") as ps0:
            wcat_ps = ps0.tile([D, 2 * D], F32, space="PSUM")
            nc.tensor.matmul(out=wcat_ps[:, 0:D], lhsT=wlt_sb[:], rhs=wk_sb[:],
                             start=True, stop=True)
            nc.tensor.matmul(out=wcat_ps[:, D:2 * D], lhsT=wlt_sb[:], rhs=wv_sb[:],
                             start=True, stop=True)
            nc.vector.tensor_copy(w_cat[:], wcat_ps[:])
            wlq_ps = ps0.tile([D, D], F32, space="PSUM")
            nc.tensor.matmul(out=wlq_ps[:], lhsT=wlt_sb[:], rhs=wq_sb[:],
                             start=True, stop=True)
            nc.scalar.mul(out=w_lq[:], in_=wlq_ps[:], mul=1.0 / math.sqrt(DH))
            wek_ps = ps0.tile([ED, D], F32, space="PSUM")
            nc.tensor.matmul(out=wek_ps[:], lhsT=wet_sb[:], rhs=wk_sb[:],
                             start=True, stop=True)
            nc.vector.tensor_copy(w_ek[:], wek_ps[:])

        # ---- phase A: build KV / KE / q tables ------------------------------
        with tc.tile_pool(name="sbA", bufs=4) as sbA, \
             tc.tile_pool(name="psA", bufs=4, space="PSUM") as psA:
            for i in range(NTX):
                xt = sbA.tile([P128, P128], F32, tag="xt")
                nc.sync.dma_start(out=xt[:], in_=xT[:, i * P128:(i + 1) * P128])
                mm = psA.tile([P128, 2 * D], F32, space="PSUM", tag="psa")
                nc.tensor.matmul(out=mm[:], lhsT=xt[:], rhs=w_cat[:],
                                 start=True, stop=True)
                kv_sb = sbA.tile([P128, 2 * D], F32, tag="kv_sb")
                nc.scalar.copy(out=kv_sb[:, 0:D], in_=mm[:, 0:D])
                nc.vector.tensor_add(kv_sb[:, D:2 * D], mm[:, D:2 * D], bv_sb[:])
                nc.sync.dma_start(out=kv_d[i * P128:(i + 1) * P128, :], in_=kv_sb[:])

            for i in range(NTE):
                et = sbA.tile([ED, P128], F32, tag="et")
                nc.sync.dma_start(out=et[:], in_=eaT[:, i * P128:(i + 1) * P128])
                kem_full = psA.tile([P128, 2 * D], F32, space="PSUM", tag="psa")
                kem = kem_full[:, 0:D]
                nc.tensor.matmul(out=kem[:], lhsT=et[:], rhs=w_ek[:],
                                 start=True, stop=True)
                ke_sb = sbA.tile([P128, D], F32, tag="ke_sb")
                nc.vector.tensor_add(ke_sb[:], kem[:], bk_sb[:])
                nc.sync.dma_start(out=ke_d[i * P128:(i + 1) * P128, :], in_=ke_sb[:])

            for i in range(NTQ):
                xq = sbA.tile([P128, P128], F32, tag="xq")
                nc.sync.dma_start(out=xq[:], in_=xqT[:, i * P128:(i + 1) * P128])
                qm_full = psA.tile([P128, 2 * D], F32, space="PSUM", tag="psa")
                qm = qm_full[:, 0:D]
                nc.tensor.matmul(out=qm[:], lhsT=xq[:], rhs=w_lq[:],
                                 start=True, stop=True)
                q_sb = sbA.tile([P128, D], F32, tag="q_sb")
                nc.vector.tensor_add(q_sb[:], qm[:], bq_sb[:])
                nc.sync.dma_start(out=q_d[i * P128:(i + 1) * P128, :], in_=q_sb[:])

        tc.strict_bb_all_engine_barrier()

        # ---- phase B: per-block gather + attention + segment reduce ---------
        with tc.tile_pool(name="sbB", bufs=4) as sbB, \
             tc.tile_pool(name="sbB2", bufs=3) as sbB2, \
             tc.tile_pool(name="ps_ob", bufs=2, space="PSUM") as ps_ob, \
             tc.tile_pool(name="ps_q", bufs=2, space="PSUM") as ps_q, \
             tc.tile_pool(name="ps_acc", bufs=2, space="PSUM") as ps_acc, \
             tc.tile_pool(name="ps_epi", bufs=2, space="PSUM") as ps_epi:
            for b in range(nblk):
                ints_sb = sbB2.tile([P128, 17], I32, tag="ints")
                nc.sync.dma_start(out=ints_sb[:], in_=ints_p[b, :, :])
                floats_sb = sbB2.tile([P128, 16], F32, tag="floats")
                nc.sync.dma_start(out=floats_sb[:], in_=floats_p[b, :, :])
                orow_sb = sbB2.tile([1, CAP], F32, tag="orow")
                nc.sync.dma_start(out=orow_sb[:], in_=orow_p[b, :, :])
                qblk = sbB2.tile([P128, D], F32, tag="qblk")
                nc.gpsimd.indirect_dma_start(
                    out=qblk[:], out_offset=None, in_=q_d[:, :],
                    in_offset=bass.IndirectOffsetOnAxis(ap=ints_sb[:, 16:17], axis=0))

                acc = ps_acc.tile([P128, D + H], F32, space="PSUM", tag="acc")
                for t in range(TPB):
                    kv_g = sbB.tile([P128, 2 * D], F32, tag="kv_g")
                    nc.gpsimd.indirect_dma_start(
                        out=kv_g[:], out_offset=None, in_=kv_d[:, :],
                        in_offset=bass.IndirectOffsetOnAxis(ap=ints_sb[:, t:t + 1], axis=0))
                    ke_g = sbB.tile([P128, D], F32, tag="ke_g")
                    nc.gpsimd.indirect_dma_start(
                        out=ke_g[:], out_offset=None, in_=ke_d[:, :],
                        in_offset=bass.IndirectOffsetOnAxis(ap=ints_sb[:, 8 + t:9 + t], axis=0))

                    Mt = sbB.tile([P128, P128], F32, tag="Mt")
                    nc.vector.tensor_tensor(
                        out=Mt[:], in0=floats_sb[:, t:t + 1].to_broadcast([P128, P128]),
                        in1=iota_row[:], op=mybir.AluOpType.is_equal)
                    ob_ps = ps_ob.tile([P128, P128], F32, space="PSUM", tag="ob")
                    nc.tensor.matmul(out=ob_ps[:], lhsT=ones_row[:],
                                     rhs=orow_sb[:, t * P128:(t + 1) * P128],
                                     start=True, stop=True)
                    ob_sb = sbB.tile([P128, P128], F32, tag="ob_sb")
                    nc.scalar.copy(out=ob_sb[:], in_=ob_ps[:])
                    MTt = sbB.tile([P128, P128], F32, tag="MTt")
                    nc.vector.tensor_tensor(
                        out=MTt[:], in0=ob_sb[:],
                        in1=iota_col[:, :1].to_broadcast([P128, P128]),
                        op=mybir.AluOpType.is_equal)
                    qx_ps = ps_q.tile([P128, D], F32, space="PSUM", tag="qx")
                    nc.tensor.matmul(out=qx_ps[:], lhsT=MTt[:], rhs=qblk[:],
                                     start=True, stop=True)

                    kk = sbB.tile([P128, D], F32, tag="kk")
                    nc.vector.tensor_add(kk[:], kv_g[:, 0:D], ke_g[:])
                    prod = sbB.tile([P128, D], F32, tag="prod")
                    nc.vector.tensor_mul(prod[:], kk[:], qx_ps[:])
                    scores = sbB.tile([P128, H], F32, tag="scores")
                    nc.vector.tensor_reduce(
                        out=scores[:], in_=prod[:].rearrange("p (h d) -> p h d", h=H),
                        axis=mybir.AxisListType.X, op=mybir.AluOpType.add)
                    wex = sbB.tile([P128, D + H], F32, tag="wex")
                    ex = wex[:, D:D + H]
                    nc.scalar.activation(
                        out=ex, in_=scores[:], func=mybir.ActivationFunctionType.Exp,
                        bias=floats_sb[:, 8 + t:9 + t], scale=1.0)
                    nc.vector.tensor_tensor(
                        out=wex[:, 0:D].rearrange("p (h d) -> p h d", h=H),
                        in0=kv_g[:, D:2 * D].rearrange("p (h d) -> p h d", h=H),
                        in1=ex.rearrange("p (h o) -> p h o", o=1).to_broadcast([P128, H, DH]),
                        op=mybir.AluOpType.mult)

                    # single matmul per tile: PSUM `start` clears has_written
                    # bank-wide, so NUM and DEN must accumulate as one group
                    nc.tensor.matmul(out=acc[:], lhsT=Mt[:], rhs=wex[:],
                                     start=(t == 0), stop=(t == TPB - 1))

                den = sbB2.tile([P128, H], F32, tag="den")
                nc.vector.tensor_scalar_add(out=den[:], in0=acc[:, D:D + H],
                                            scalar1=1e-30)
                denr = sbB2.tile([P128, H], F32, tag="denr")
                nc.vector.reciprocal(denr[:], den[:])
                ctx_sb = sbB2.tile([P128, D], F32, tag="ctx")
                nc.vector.tensor_tensor(
                    out=ctx_sb[:].rearrange("p (h d) -> p h d", h=H),
                    in0=acc[:, 0:D].rearrange("p (h d) -> p h d", h=H),
                    in1=denr[:].rearrange("p (h o) -> p h o", o=1).to_broadcast([P128, H, DH]),
                    op=mybir.AluOpType.mult)
                ctxT_ps = ps_epi.tile([P128, D], F32, space="PSUM", tag="epi")
                nc.tensor.transpose(out=ctxT_ps[:], in_=ctx_sb[:], identity=ident[:])
                ctxT_sb = sbB2.tile([P128, D], F32, tag="ctxT_sb")
                nc.scalar.copy(out=ctxT_sb[:], in_=ctxT_ps[:])
                o_ps = ps_epi.tile([P128, D], F32, space="PSUM", tag="epi")
                nc.tensor.matmul(out=o_ps[:], lhsT=ctxT_sb[:], rhs=wo_sb[:],
                                 start=True, stop=True)
                o_sb = sbB2.tile([P128, D], F32, tag="o_sb")
                nc.vector.tensor_add(o_sb[:], o_ps[:], bo_sb[:])
                o_relu = sbB2.tile([P128, D], F32, tag="o_relu")
                nc.scalar.activation(out=o_relu[:], in_=o_sb[:],
                                     func=mybir.ActivationFunctionType.Relu)
                nc.sync.dma_start(out=out[b * P128:(b + 1) * P128, :], in_=o_relu[:])

    nc.compile()
    return nc


_CACHE = {}


def _get_nc(nblk):
    if nblk not in _CACHE:
        _CACHE[nblk] = _build_nc(nblk)
    return _CACHE[nblk]


def kernel(**inputs):
    x = np.ascontiguousarray(np.asarray(inputs["x"], np.float32))
    ea = np.ascontiguousarray(np.asarray(inputs["edge_attr"], np.float32))
    owners = np.asarray(inputs["owners"], np.int32)
    pair_e = np.asarray(inputs["pair_e"], np.int32)
    pair_u = np.asarray(inputs["pair_u"], np.int32)

    nblk, per_core, node_map = _route(owners, pair_e, pair_u)
    nc = _get_nc(nblk)

    xT = np.zeros((P128, PADN), np.float32)
    xT[:, :N] = x.T
    eaT = np.zeros((ED, PADE), np.float32)
    eaT[:, :E] = ea.T

    def trep(b):
        return np.tile(np.asarray(b, np.float32)[None, :], (P128, 1))

    shared = dict(
        xT=xT, eaT=eaT,
        w_linT=np.ascontiguousarray(np.asarray(inputs["w_lin"], np.float32).T),
        w_eT=np.ascontiguousarray(np.asarray(inputs["w_e"], np.float32).T),
        w_q=np.asarray(inputs["w_q"], np.float32),
        w_k=np.asarray(inputs["w_k"], np.float32),
        w_v=np.asarray(inputs["w_v"], np.float32),
        w_o=np.asarray(inputs["w_o"], np.float32),
        bk_m=trep(inputs["b_k"]), bv_m=trep(inputs["b_v"]),
        bq_m=trep(inputs["b_q"]), bo_m=trep(inputs["b_o"]),
    )
    in_maps = []
    for c in range(NC):
        ints, floats, orow = per_core[c]
        xqT = np.zeros((P128, PADQ), np.float32)
        xqT[:, :NLOC] = x[c * NLOC:(c + 1) * NLOC].T
        in_maps.append(dict(shared, xqT=xqT, ints_p=ints, floats_p=floats,
                            orow_p=orow))

    import os
    trace = os.environ.get("KERNEL_TRACE", "0") == "1"
    kwargs = {}
    if trace:
        kwargs = dict(trace=True, tmpdir=os.environ.get("KERNEL_TRACE_DIR") or None)
    res = run_bass_kernel_spmd(nc, in_maps, core_ids=list(range(NC)), **kwargs)
    global _LAST_RESULTS
    _LAST_RESULTS = res

    out_full = np.zeros((N, D), np.float32)
    for c in range(NC):
        oc = res.results[c]["out"]
        valid = node_map[c] >= 0
        out_full[node_map[c][valid]] = oc[valid]
    return out_full


# revision 8
# speedup vs baseline: 1.6148x; 1.6148x over previous
"""Trainium2 Bass kernel for CustomHyperSemanticMessagePassing (hypergraph
multi-head single-query attention message passing).

Math (reference):
  Wh = x @ w_lin ; We = edge_attr @ w_e
  q  = (Wh @ w_q + b_q)/sqrt(dh)               per node, [N,H,dh]
  k_p = (Wh[u] + We[e]) @ w_k + b_k            per pair (v,e,u)
  v_p = Wh[u] @ w_v + b_v
  scores_p = <q[v], k_p> per head ; segmented softmax over each node v's pairs
  out = relu(segsum(alpha * v_p) @ w_o + b_o)

Kernel strategy (8 NeuronCores, SPMD, no collectives):
  * Algebraic refactor: fold w_lin into the q/k/v projections so no per-pair
    matmuls are needed:
      KV table  [N,256] = [x @ (w_lin w_k) | x @ (w_lin w_v) + b_v]
      KE table  [E,128] =  edge_attr @ (w_e w_k) + b_k
      q table   [slots,128] = (x_perm @ (w_lin w_q) + b_q)/4, block-ordered
    k_p = KV[u,:128] + KE[e]; v_p = KV[u,128:].
  * Every core builds the full KV/KE tables (the tables must land in every
    core's HBM anyway; recompute is cheaper than collectives) and the q table
    for its own 1/8 node slab, with q rows pre-permuted to block order.
  * Pairs are routed (host side) to the core owning v, sorted by owner, and
    bin-packed into blocks of <=128 owner nodes / <=1024 pairs. Per 128-pair
    tile the device gathers KV/KE rows with indirect DMA, expands q via a
    one-hot matmul (one-hot built with iota+is_equal, transposed on PE),
    computes scores with vector ops, exp on ScalarE (scores are O(1),
    max-subtraction provably unnecessary), and segment-reduces
    numerator+denominator with a single one-hot scatter matmul accumulated in
    PSUM across the block's 8 tiles.
  * One-hot matrices are exact in bf16, so the expansion/scatter matmuls run
    in bf16 (single-pass on the fp32-double-pumped PE); accumulation stays
    fp32 in PSUM.
  * Host un-permutes the per-core block-major outputs into the final [N,128].
"""

import math
import numpy as np
from contextlib import ExitStack

import concourse.bass as bass
import concourse.bacc as bacc
import concourse.tile as tile
import concourse.mybir as mybir
from concourse.bass_utils import run_bass_kernel_spmd
from concourse.masks import make_identity

F32 = mybir.dt.float32
BF16 = mybir.dt.bfloat16
I32 = mybir.dt.int32

N, E, D, ED, H = 100000, 50000, 128, 64, 8
DH = D // H
NC = 8
NLOC = N // NC
P128 = 128
TPB = 8                  # 128-pair tiles per block
CAP = TPB * P128         # pairs per block
CAP_NODES = 128          # owner nodes per block (M-matrix columns)
PAD_BIAS = -30000.0      # additive score bias for padding pairs -> exp == 0
ABATCH = 8               # phase-A tiles per DMA batch

NTX = -(-N // P128)                  # 782 x tiles
NTX4 = -(-NTX // ABATCH) * ABATCH    # padded to batch -> 784
PADN = NTX4 * P128
NTE = -(-E // P128)                  # 391 edge tiles
NTE4 = -(-NTE // ABATCH) * ABATCH    # 392
PADE = NTE4 * P128


# ----------------------------------------------------------------------------
# host-side routing
# ----------------------------------------------------------------------------

def _pack_nodes(cnt, nblk):
    """Worst-fit-decreasing packing of nodes into nblk blocks with
    <=CAP_NODES nodes and <=CAP pairs each. Returns list of node-id lists or
    None if it does not fit."""
    import heapq
    order = np.argsort(-cnt, kind="stable")
    heap = [(-CAP, bi) for bi in range(nblk)]
    heapq.heapify(heap)
    nodes = [[] for _ in range(nblk)]
    for nid in order:
        c = int(cnt[nid])
        placed = False
        while heap:
            negrem, bi = heapq.heappop(heap)
            rem = -negrem
            if rem < c:
                heapq.heappush(heap, (negrem, bi))
                break
            nodes[bi].append(nid)
            if len(nodes[bi]) < CAP_NODES:
                heapq.heappush(heap, (-(rem - c), bi))
            placed = True
            break
        if not placed:
            return None
    return nodes


def _route(owners, pair_e, pair_u):
    """Sort pairs by owner, split per core, pack blocks, build device arrays.

    Returns (nblk, per_core_arrays, node_map). per_core_arrays[c] =
    (ints [nblk,128,16]  : pair_u cols 0..7, pair_e cols 8..15, by tile,
     floats [nblk,128,16]: owner col-index cols 0..7, pad bias cols 8..15,
     qcols [nblk*128]    : global x row for each block column, -1 for pad).
    node_map[c] maps output row -> global node id (-1 for padding)."""
    perm = np.argsort(owners, kind="stable")
    o_s = owners[perm]
    e_s = pair_e[perm]
    u_s = pair_u[perm]
    bounds = np.searchsorted(o_s, np.arange(NC + 1) * NLOC)

    packs = []
    nblk = 0
    for c in range(NC):
        lo, hi = int(bounds[c]), int(bounds[c + 1])
        loc = o_s[lo:hi] - c * NLOC
        cnt = np.bincount(loc, minlength=NLOC)
        starts = np.zeros(NLOC + 1, np.int64)
        np.cumsum(cnt, out=starts[1:])
        nb = max(math.ceil(NLOC / CAP_NODES), math.ceil((hi - lo) / CAP))
        while True:
            nodes = _pack_nodes(cnt, nb)
            if nodes is not None:
                break
            nb += 1
        packs.append((lo, nodes, cnt, starts))
        nblk = max(nblk, nb)

    per_core = []
    maps = []
    for c in range(NC):
        lo, nodes, cnt, starts = packs[c]
        ints = np.zeros((nblk, P128, 16), np.int32)
        floats = np.zeros((nblk, P128, 16), np.float32)
        orows = np.zeros((nblk, 1, CAP), np.float32)
        qcols = np.full(nblk * P128, -1, np.int64)
        nmap = np.full(nblk * P128, -1, np.int64)
        for b, blk in enumerate(nodes):
            fu = np.zeros(CAP, np.int32)
            fe = np.zeros(CAP, np.int32)
            frel = np.zeros(CAP, np.float32)
            fbias = np.full(CAP, PAD_BIAS, np.float32)
            pos = 0
            for j, nid in enumerate(blk):
                s0 = lo + int(starts[nid])
                k = int(cnt[nid])
                fu[pos:pos + k] = u_s[s0:s0 + k]
                fe[pos:pos + k] = e_s[s0:s0 + k]
                frel[pos:pos + k] = j
                fbias[pos:pos + k] = 0.0
                qcols[b * P128 + j] = c * NLOC + nid
                nmap[b * P128 + j] = c * NLOC + nid
                pos += k
            ints[b, :, 0:8] = fu.reshape(TPB, P128).T
            ints[b, :, 8:16] = fe.reshape(TPB, P128).T
            floats[b, :, 0:8] = frel.reshape(TPB, P128).T
            floats[b, :, 8:16] = fbias.reshape(TPB, P128).T
            orows[b, 0, :] = frel
        per_core.append((ints, floats, qcols, orows))
        maps.append(nmap)
    return nblk, per_core, np.stack(maps)


# ----------------------------------------------------------------------------
# device program
# ----------------------------------------------------------------------------

def _build_nc(nblk):
    nc = bacc.Bacc()
    ntq = -(-nblk // ABATCH) * ABATCH      # q tiles (one per block), padded
    padq = ntq * P128

    xT = nc.declare_dram_parameter("xT", [P128, PADN], BF16, isOutput=False)
    xqT = nc.declare_dram_parameter("xqT", [P128, padq], BF16, isOutput=False)
    eaT = nc.declare_dram_parameter("eaT", [ED, PADE], BF16, isOutput=False)
    w_linT = nc.declare_dram_parameter("w_linT", [D, D], F32, isOutput=False)
    w_eT = nc.declare_dram_parameter("w_eT", [D, ED], F32, isOutput=False)
    w_q = nc.declare_dram_parameter("w_q", [D, D], F32, isOutput=False)
    w_k = nc.declare_dram_parameter("w_k", [D, D], F32, isOutput=False)
    w_v = nc.declare_dram_parameter("w_v", [D, D], F32, isOutput=False)
    w_o = nc.declare_dram_parameter("w_o", [D, D], F32, isOutput=False)
    bcat_m = nc.declare_dram_parameter("bcat_m", [P128, 2 * D], F32, isOutput=False)
    bk_m = nc.declare_dram_parameter("bk_m", [P128, D], F32, isOutput=False)
    bq_m = nc.declare_dram_parameter("bq_m", [P128, D], F32, isOutput=False)
    bo_m = nc.declare_dram_parameter("bo_m", [P128, D], F32, isOutput=False)
    ints_p = nc.declare_dram_parameter("ints_p", [nblk, P128, 16], I32, isOutput=False)
    floats_p = nc.declare_dram_parameter("floats_p", [nblk, P128, 16], F32, isOutput=False)
    orow_p = nc.declare_dram_parameter("orow_p", [nblk, 1, CAP], BF16, isOutput=False)
    out = nc.declare_dram_parameter("out", [nblk * P128, D], F32, isOutput=True)

    kv_d = nc.dram_tensor("kv_d", [PADN, 2 * D], BF16)
    ke_d = nc.dram_tensor("ke_d", [PADE, D], BF16)
    q_d = nc.dram_tensor("q_d", [padq, D], BF16)

    with ExitStack() as ctx:
        tc = ctx.enter_context(tile.TileContext(nc))
        consts = ctx.enter_context(tc.tile_pool(name="consts", bufs=1))

        ident_bf = consts.tile([P128, P128], BF16)
        make_identity(nc, ident_bf[:])
        ident_f = consts.tile([P128, P128], F32)
        make_identity(nc, ident_f[:])
        iota_row_i = consts.tile([P128, P128], I32)
        nc.gpsimd.iota(iota_row_i[:], pattern=[[1, P128]], base=0, channel_multiplier=0)
        iota_row = consts.tile([P128, P128], F32)
        nc.vector.tensor_copy(iota_row[:], iota_row_i[:])
        iota_col_i = consts.tile([P128, 1], I32)
        nc.gpsimd.iota(iota_col_i[:], pattern=[[0, 1]], base=0, channel_multiplier=1)
        iota_col = consts.tile([P128, 1], F32)
        nc.vector.tensor_copy(iota_col[:], iota_col_i[:])
        ones_row = consts.tile([1, P128], BF16)
        nc.vector.memset(ones_row[:], 1.0)

        # ---- load weights / fold projections --------------------------------
        wlt_sb = consts.tile([D, D], F32)
        nc.sync.dma_start(out=wlt_sb[:], in_=w_linT[:, :])
        wet_sb = consts.tile([D, ED], F32)
        nc.sync.dma_start(out=wet_sb[:], in_=w_eT[:, :])
        wq_sb = consts.tile([D, D], F32)
        nc.sync.dma_start(out=wq_sb[:], in_=w_q[:, :])
        wk_sb = consts.tile([D, D], F32)
        nc.sync.dma_start(out=wk_sb[:], in_=w_k[:, :])
        wv_sb = consts.tile([D, D], F32)
        nc.sync.dma_start(out=wv_sb[:], in_=w_v[:, :])
        wo_sb = consts.tile([D, D], F32)
        nc.sync.dma_start(out=wo_sb[:], in_=w_o[:, :])
        bcat_sb = consts.tile([P128, 2 * D], F32)
        nc.sync.dma_start(out=bcat_sb[:], in_=bcat_m[:, :])
        bk_sb = consts.tile([P128, D], F32)
        nc.sync.dma_start(out=bk_sb[:], in_=bk_m[:, :])
        bq_raw = consts.tile([P128, D], F32)
        nc.sync.dma_start(out=bq_raw[:], in_=bq_m[:, :])
        bo_sb = consts.tile([P128, D], F32)
        nc.sync.dma_start(out=bo_sb[:], in_=bo_m[:, :])
        bq_sb = consts.tile([P128, D], F32)
        nc.scalar.mul(out=bq_sb[:], in_=bq_raw[:], mul=1.0 / math.sqrt(DH))

        w_cat = consts.tile([D, 2 * D], BF16)     # [w_lin w_k | w_lin w_v]
        w_lq = consts.tile([D, D], BF16)          # (w_lin w_q)/sqrt(dh)
        w_ek = consts.tile([ED, D], BF16)         # w_e w_k

        with tc.tile_pool(name="ps0", bufs=1, space="PSUM") as ps0:
            wcat_ps = ps0.tile([D, 2 * D], F32, space="PSUM")
            nc.tensor.matmul(out=wcat_ps[:, 0:D], lhsT=wlt_sb[:], rhs=wk_sb[:],
                             start=True, stop=True)
            nc.tensor.matmul(out=wcat_ps[:, D:2 * D], lhsT=wlt_sb[:], rhs=wv_sb[:],
                             start=True, stop=True)
            nc.vector.tensor_copy(w_cat[:], wcat_ps[:])
            wlq_ps = ps0.tile([D, D], F32, space="PSUM")
            nc.tensor.matmul(out=wlq_ps[:], lhsT=wlt_sb[:], rhs=wq_sb[:],
                             start=True, stop=True)
            nc.scalar.mul(out=w_lq[:], in_=wlq_ps[:], mul=1.0 / math.sqrt(DH))
            wek_ps = ps0.tile([ED, D], F32, space="PSUM")
            nc.tensor.matmul(out=wek_ps[:], lhsT=wet_sb[:], rhs=wk_sb[:],
                             start=True, stop=True)
            nc.vector.tensor_copy(w_ek[:], wek_ps[:])

        # ---- phase A: build KV / KE / q tables (4x-batched DMA) -------------
        with tc.tile_pool(name="sbA", bufs=4) as sbA, \
             tc.tile_pool(name="psA", bufs=4, space="PSUM") as psA:
            for i in range(NTX4 // ABATCH):
                xt4 = sbA.tile([P128, ABATCH * P128], BF16, tag="xt4")
                nc.sync.dma_start(
                    out=xt4[:], in_=xT[:, i * ABATCH * P128:(i + 1) * ABATCH * P128])
                kv4 = sbA.tile([P128, ABATCH, 2 * D], BF16, tag="kv4")
                for k in range(ABATCH):
                    mm = psA.tile([P128, 2 * D], F32, space="PSUM", tag="psa")
                    nc.tensor.matmul(out=mm[:], lhsT=xt4[:, k * P128:(k + 1) * P128],
                                     rhs=w_cat[:], start=True, stop=True)
                    nc.vector.tensor_add(kv4[:, k, :], mm[:], bcat_sb[:])
                dst = kv_d[i * ABATCH * P128:(i + 1) * ABATCH * P128, :]
                nc.sync.dma_start(
                    out=dst.rearrange("(k p) w -> p k w", p=P128), in_=kv4[:])

            for i in range(NTE4 // ABATCH):
                et4 = sbA.tile([ED, ABATCH * P128], BF16, tag="et4")
                nc.sync.dma_start(
                    out=et4[:], in_=eaT[:, i * ABATCH * P128:(i + 1) * ABATCH * P128])
                ke4 = sbA.tile([P128, ABATCH, D], BF16, tag="ke4")
                for k in range(ABATCH):
                    kem_full = psA.tile([P128, 2 * D], F32, space="PSUM", tag="psa")
                    kem = kem_full[:, 0:D]
                    nc.tensor.matmul(out=kem, lhsT=et4[:, k * P128:(k + 1) * P128],
                                     rhs=w_ek[:], start=True, stop=True)
                    nc.vector.tensor_add(ke4[:, k, :], kem, bk_sb[:])
                dst = ke_d[i * ABATCH * P128:(i + 1) * ABATCH * P128, :]
                nc.sync.dma_start(
                    out=dst.rearrange("(k p) w -> p k w", p=P128), in_=ke4[:])

            for i in range(ntq // ABATCH):
                xq4 = sbA.tile([P128, ABATCH * P128], BF16, tag="xq4")
                nc.sync.dma_start(
                    out=xq4[:], in_=xqT[:, i * ABATCH * P128:(i + 1) * ABATCH * P128])
                q4 = sbA.tile([P128, ABATCH, D], BF16, tag="q4")
                for k in range(ABATCH):
                    qm_full = psA.tile([P128, 2 * D], F32, space="PSUM", tag="psa")
                    qm = qm_full[:, 0:D]
                    nc.tensor.matmul(out=qm, lhsT=xq4[:, k * P128:(k + 1) * P128],
                                     rhs=w_lq[:], start=True, stop=True)
                    nc.vector.tensor_add(q4[:, k, :], qm, bq_sb[:])
                dst = q_d[i * ABATCH * P128:(i + 1) * ABATCH * P128, :]
                nc.sync.dma_start(
                    out=dst.rearrange("(k p) w -> p k w", p=P128), in_=q4[:])

        tc.strict_bb_all_engine_barrier()

        # ---- phase B: per-block gather + attention + segment reduce ---------
        with tc.tile_pool(name="sbB", bufs=6) as sbB, \
             tc.tile_pool(name="sbB2", bufs=4) as sbB2, \
             tc.tile_pool(name="ps_mt", bufs=2, space="PSUM") as ps_mt, \
             tc.tile_pool(name="ps_q", bufs=2, space="PSUM") as ps_q, \
             tc.tile_pool(name="ps_acc", bufs=2, space="PSUM") as ps_acc, \
             tc.tile_pool(name="ps_epi", bufs=2, space="PSUM") as ps_epi:
            for b in range(nblk):
                ints_sb = sbB2.tile([P128, 16], I32, tag="ints")
                nc.sync.dma_start(out=ints_sb[:], in_=ints_p[b, :, :])
                floats_sb = sbB2.tile([P128, 16], F32, tag="floats")
                nc.sync.dma_start(out=floats_sb[:], in_=floats_p[b, :, :])
                qblk = sbB2.tile([P128, D], BF16, tag="qblk")
                nc.sync.dma_start(out=qblk[:], in_=q_d[b * P128:(b + 1) * P128, :])
                orow_sb = sbB2.tile([1, CAP], BF16, tag="orow")
                nc.sync.dma_start(out=orow_sb[:], in_=orow_p[b, :, :])

                acc = ps_acc.tile([P128, D + H], F32, space="PSUM", tag="acc")
                for t in range(TPB):
                    kv_g = sbB.tile([P128, 2 * D], BF16, tag="kv_g")
                    nc.gpsimd.indirect_dma_start(
                        out=kv_g[:], out_offset=None, in_=kv_d[:, :],
                        in_offset=bass.IndirectOffsetOnAxis(ap=ints_sb[:, t:t + 1], axis=0))
                    ke_g = sbB.tile([P128, D], BF16, tag="ke_g")
                    nc.gpsimd.indirect_dma_start(
                        out=ke_g[:], out_offset=None, in_=ke_d[:, :],
                        in_offset=bass.IndirectOffsetOnAxis(ap=ints_sb[:, 8 + t:9 + t], axis=0))

                    Mt = sbB.tile([P128, P128], BF16, tag="Mt")
                    nc.vector.tensor_tensor(
                        out=Mt[:], in0=floats_sb[:, t:t + 1].to_broadcast([P128, P128]),
                        in1=iota_row[:], op=mybir.AluOpType.is_equal)
                    ob_ps = ps_mt.tile([P128, P128], F32, space="PSUM", tag="mtp")
                    nc.tensor.matmul(out=ob_ps[:], lhsT=ones_row[:],
                                     rhs=orow_sb[:, t * P128:(t + 1) * P128],
                                     start=True, stop=True)
                    MTt = sbB.tile([P128, P128], BF16, tag="MTt")
                    nc.vector.tensor_tensor(
                        out=MTt[:], in0=ob_ps[:],
                        in1=iota_col[:, :1].to_broadcast([P128, P128]),
                        op=mybir.AluOpType.is_equal)
                    qx_ps = ps_q.tile([P128, D], F32, space="PSUM", tag="qx")
                    nc.tensor.matmul(out=qx_ps[:], lhsT=MTt[:], rhs=qblk[:],
                                     start=True, stop=True)

                    kk = sbB.tile([P128, D], BF16, tag="kk")
                    nc.vector.tensor_add(kk[:], kv_g[:, 0:D], ke_g[:])
                    prod = sbB.tile([P128, D], F32, tag="prod")
                    nc.vector.tensor_mul(prod[:], kk[:], qx_ps[:])
                    scores = sbB.tile([P128, H], F32, tag="scores")
                    nc.vector.tensor_reduce(
                        out=scores[:], in_=prod[:].rearrange("p (h d) -> p h d", h=H),
                        axis=mybir.AxisListType.X, op=mybir.AluOpType.add)
                    wex = sbB.tile([P128, D + H], BF16, tag="wex")
                    ex = wex[:, D:D + H]
                    nc.scalar.activation(
                        out=ex, in_=scores[:], func=mybir.ActivationFunctionType.Exp,
                        bias=floats_sb[:, 8 + t:9 + t], scale=1.0)
                    nc.vector.tensor_tensor(
                        out=wex[:, 0:D].rearrange("p (h d) -> p h d", h=H),
                        in0=kv_g[:, D:2 * D].rearrange("p (h d) -> p h d", h=H),
                        in1=ex.rearrange("p (h o) -> p h o", o=1).to_broadcast([P128, H, DH]),
                        op=mybir.AluOpType.mult)

                    # single matmul per tile: PSUM `start` clears has_written
                    # bank-wide, so NUM and DEN must accumulate as one group
                    nc.tensor.matmul(out=acc[:], lhsT=Mt[:], rhs=wex[:],
                                     start=(t == 0), stop=(t == TPB - 1))

                den = sbB2.tile([P128, H], F32, tag="den")
                nc.vector.tensor_scalar_add(out=den[:], in0=acc[:, D:D + H],
                                            scalar1=1e-30)
                denr = sbB2.tile([P128, H], F32, tag="denr")
                nc.vector.reciprocal(denr[:], den[:])
                ctx_sb = sbB2.tile([P128, D], F32, tag="ctx")
                nc.vector.tensor_tensor(
                    out=ctx_sb[:].rearrange("p (h d) -> p h d", h=H),
                    in0=acc[:, 0:D].rearrange("p (h d) -> p h d", h=H),
                    in1=denr[:].rearrange("p (h o) -> p h o", o=1).to_broadcast([P128, H, DH]),
                    op=mybir.AluOpType.mult)
                ctxT_ps = ps_epi.tile([P128, D], F32, space="PSUM", tag="epi")
                nc.tensor.transpose(out=ctxT_ps[:], in_=ctx_sb[:], identity=ident_f[:])
                ctxT_sb = sbB2.tile([P128, D], F32, tag="ctxT_sb")
                nc.scalar.copy(out=ctxT_sb[:], in_=ctxT_ps[:])
                o_ps = ps_epi.tile([P128, D], F32, space="PSUM", tag="epi")
                nc.tensor.matmul(out=o_ps[:], lhsT=ctxT_sb[:], rhs=wo_sb[:],
                                 start=True, stop=True)
                o_sb = sbB2.tile([P128, D], F32, tag="o_sb")
                nc.vector.tensor_add(o_sb[:], o_ps[:], bo_sb[:])
                o_relu = sbB2.tile([P128, D], F32, tag="o_relu")
                nc.scalar.activation(out=o_relu[:], in_=o_sb[:],
                                     func=mybir.ActivationFunctionType.Relu)
                nc.sync.dma_start(out=out[b * P128:(b + 1) * P128, :], in_=o_relu[:])

    nc.compile()
    return nc


_CACHE = {}


def _get_nc(nblk):
    if nblk not in _CACHE:
        _CACHE[nblk] = _build_nc(nblk)
    return _CACHE[nblk]


def kernel(**inputs):
    import ml_dtypes
    x = np.ascontiguousarray(np.asarray(inputs["x"], np.float32))
    ea = np.ascontiguousarray(np.asarray(inputs["edge_attr"], np.float32))
    owners = np.asarray(inputs["owners"], np.int32)
    pair_e = np.asarray(inputs["pair_e"], np.int32)
    pair_u = np.asarray(inputs["pair_u"], np.int32)

    nblk, per_core, node_map = _route(owners, pair_e, pair_u)
    nc = _get_nc(nblk)
    ntq = -(-nblk // ABATCH) * ABATCH
    padq = ntq * P128

    bf = ml_dtypes.bfloat16
    xT = np.zeros((P128, PADN), bf)
    xT[:, :N] = x.T.astype(bf)
    eaT = np.zeros((ED, PADE), bf)
    eaT[:, :E] = ea.T.astype(bf)

    def trep(b):
        return np.tile(np.asarray(b, np.float32)[None, :], (P128, 1))

    bcat = np.concatenate(
        [np.zeros(D, np.float32), np.asarray(inputs["b_v"], np.float32)])
    shared = dict(
        xT=xT, eaT=eaT,
        w_linT=np.ascontiguousarray(np.asarray(inputs["w_lin"], np.float32).T),
        w_eT=np.ascontiguousarray(np.asarray(inputs["w_e"], np.float32).T),
        w_q=np.asarray(inputs["w_q"], np.float32),
        w_k=np.asarray(inputs["w_k"], np.float32),
        w_v=np.asarray(inputs["w_v"], np.float32),
        w_o=np.asarray(inputs["w_o"], np.float32),
        bcat_m=trep(bcat),
        bk_m=trep(inputs["b_k"]),
        bq_m=trep(inputs["b_q"]), bo_m=trep(inputs["b_o"]),
    )
    in_maps = []
    for c in range(NC):
        ints, floats, qcols, orows = per_core[c]
        xqT = np.zeros((P128, padq), bf)
        valid = qcols >= 0
        xqT[:, np.nonzero(valid)[0]] = x[qcols[valid]].T.astype(bf)
        in_maps.append(dict(shared, xqT=xqT, ints_p=ints, floats_p=floats,
                            orow_p=orows.astype(ml_dtypes.bfloat16)))

    import os
    trace = os.environ.get("KERNEL_TRACE", "0") == "1"
    kwargs = {}
    if trace:
        kwargs = dict(trace=True, tmpdir=os.environ.get("KERNEL_TRACE_DIR") or None)
    res = run_bass_kernel_spmd(nc, in_maps, core_ids=list(range(NC)), **kwargs)
    global _LAST_RESULTS
    _LAST_RESULTS = res

    out_full = np.zeros((N, D), np.float32)
    for c in range(NC):
        oc = res.results[c]["out"]
        valid = node_map[c] >= 0
        out_full[node_map[c][valid]] = oc[valid]
    return out_full
